# revision 35
# baseline (speedup 1.0000x reference)
"""Trainium2 Bass kernel for the MLPSim adjacency-constructor problem.

Full shapes: spatial [4, 2048, 32], temporal [4, 288, 32], output
adj [4, 2336, 2336] f32 where adj = tanh(relu(blocks)):
  ss = tanh(m - m^T), m = nv1 @ nv2^T, nv_i = tanh(3*x@W_i^T)
  st = s1[n] + s2[t] + b_st ;  ts = s1t[t] + s2t[n] + b_ts
  tt = triu(temporal @ temporal^T)

Sharding: 8 cores = (batch b = c//2) x (row-half h = c%2); each core emits
1024 spatial + 144 temporal rows ([1168, 2336]) of one batch, stored fp16
on device and upcast to f32 on the host during unshard.

Device algebra (ACT is the wall: 1 elem/lane/cycle, dtype-independent):
  ss out = tanh(tanh(relu(z))), z = m - m^T. Minimax fit
    tanh(tanh(relu(z))) ~= max((C + D*y)*y, 0),  y = tanh(A*z)
  with A=1.15118303 C=0.90136458 D=-0.141975 (fit err 2.2e-3; (C+D*y)*y is
  negative whenever y<0, so the final relu also zeroes the z<0 half) folds
  the two full-size ACT tanh passes into ONE; the quadratic runs on DVE in
  3 fused 16-bit ops. z is a plain fp16 matmul (nv exact, one fp16
  rounding -> ~7e-3 total err vs the 2e-2 gate). nv pre-acts u = x@W are
  an exact bf16 hi/lo K-stacked matmul ([Wh;Wh;Wl].T @ [xh;xl;xh], err
  ~1e-5), avoiding slow f32 matmuls. st is a K=9 fp16 matmul (chunk-
  indicator rows) plus rank-1 s2 accumulate-matmuls + ONE ACT pass for
  all 8 chunks; ts is a K=2 rank-1 fp16 matmul; tt stays f32 (288 cols).
  Temporal passes interleave into the tail of the spatial ACT stream.
"""

import numpy as np
from contextlib import ExitStack

import concourse.bass as bass
from concourse import mybir
from concourse.bass_utils import run_bass_kernel_spmd

AF = mybir.ActivationFunctionType
ALU = mybir.AluOpType
F32 = mybir.dt.float32
F16 = mybir.dt.float16
BF16 = mybir.dt.bfloat16

B, N, T, D = 4, 2048, 288, 32
NS = N // 2
TS = T // 2
NT = N + T
ROWS = NS + TS
N_CORES = 8
NCHUNK = NS // 128

AA = 1.15118303
CA = 0.90136458
DA = -0.141975

G_A = ("sp96", "W96_R")
G_L = ("sp96r", "W96_L")

# packed fp16 blob layout (columns)
PK_SPT = 0            # spT16 full [32, 2048]
PK_SPTR = 2048        # spTr16 rows-half [32, 1024]
PK_TMT = 3072         # tmT16 [32, 288]
PK_TMTR = 3360        # tmTr16 [32, 144]
PK_W = 3504           # wst_a | wst_b | wts_a | wts_b, one col each
PK_B = 3508           # bst at [0, 3508], bts at [0, 3509]
PK_W16 = 3510


def build_program():
    nc = bass.Bass()
    inp = {}

    for name, shape, dt in (
        ("sp96", (3 * D, N), BF16), ("W96_R", (3 * D, 2 * D), BF16),
        ("sp96r", (3 * D, NS), BF16), ("W96_L", (3 * D, 2 * D), BF16),
        ("pk16", (D, PK_W16), F16), ("pk32", (D, T + TS + 2), F32),
        ("ttmask16", (TS, T), F16), ("stind", (9, NCHUNK * T), F16),
    ):
        inp[name] = nc.declare_dram_parameter(name, list(shape), dt, isOutput=False)
    out = nc.declare_dram_parameter("out", [ROWS, NT], F16, isOutput=True)

    ctx = ExitStack()
    _uid = [0]

    def sbuf(shape, dt=F16):
        _uid[0] += 1
        return ctx.enter_context(nc.sbuf_tensor(f"sb{_uid[0]}", list(shape), dt))

    with ctx:
        t_in = {k: sbuf(v.shape, v.dtype) for k, v in inp.items() if k != "ttmask16"}
        masks = [sbuf([128, T]), sbuf([TS - 128, T])]
        Rf16 = sbuf([2 * D, N])
        Lf16 = sbuf([2 * D, NS])
        s2row = sbuf([1, T])
        s2tb = sbuf([1, N])
        ones = sbuf([1, N])
        stL = sbuf([9, 128])
        s1row = sbuf([1, NS])
        tsL = sbuf([2, TS])           # [s1t ; ones]
        tsR = sbuf([2, N])            # [ones ; s2t+b]
        y_st = sbuf([128, NCHUNK * T])
        ybufs = [sbuf([128, N]) for _ in range(3)]
        yrbs = [sbuf([128, N]) for _ in range(2)]
        wb = sbuf([128, N])
        ytb = sbuf([128, N])
        tttb = sbuf([128, T])
        ttres = [sbuf([128, T]), sbuf([TS - 128, T])]
        scr = sbuf([1, 8], F32)
        outbufs = [sbuf([128, NT]) for _ in range(3)]

        pk = t_in["pk16"]
        spT16 = pk[:, PK_SPT:PK_SPT + N]
        spTr16 = pk[:, PK_SPTR:PK_SPTR + NS]
        tmT16 = pk[:, PK_TMT:PK_TMT + T]
        tmTr16 = pk[:, PK_TMTR:PK_TMTR + TS]
        wst_a = pk[:, PK_W:PK_W + 1]
        wst_b = pk[:, PK_W + 1:PK_W + 2]
        wts_a = pk[:, PK_W + 2:PK_W + 3]
        wts_b = pk[:, PK_W + 3:PK_W + 4]
        tmT32 = t_in["pk32"][:, 0:T]
        tmTr32 = t_in["pk32"][:, T:T + TS]
        bst_ap = t_in["pk32"][0:1, T + TS:T + TS + 1]
        bts_ap = t_in["pk32"][0:1, T + TS + 1:T + TS + 2]

        sems = {}
        for sname in ("dina", "dinl", "dinb", "dinc", "dmx", "pe_s", "act_s",
                      "dve_s", "gps_s", "dout0", "dout1", "dout2"):
            sems[sname] = ctx.enter_context(nc.semaphore(sname))
        SEM = {"pe": sems["pe_s"], "act": sems["act_s"], "dve": sems["dve_s"],
               "gps": sems["gps_s"],
               "dina": sems["dina"], "dinl": sems["dinl"], "dinb": sems["dinb"],
               "dinc": sems["dinc"], "dmx": sems["dmx"],
               "dout0": sems["dout0"], "dout1": sems["dout1"], "dout2": sems["dout2"]}

        plan = {"sync": [], "tensor": [], "scalar": [], "vector": [], "gpsimd": []}
        cnt = {k: 0 for k in SEM}

        def op(engine, waits, fn, inc=None):
            plan[engine].append((waits or [], fn, inc))
            if inc:
                cnt[inc] += 1 if inc in ("pe", "act", "dve", "gps") else 16
                return cnt[inc]
            return None

        def pe(waits, fn, inc=None):
            return op("tensor", waits, fn, inc)

        def act(waits, fn):
            return op("scalar", waits, fn, "act")

        def dve(waits, fn):
            return op("vector", waits, fn, "dve")

        def gps(waits, fn):
            return op("gpsimd", waits, fn, "gps")

        mm = nc.tensor.matmul
        act_i = nc.scalar.activation
        V = nc.vector
        G = nc.gpsimd

        # ---------- input loads, finest-grained gating first ----------
        def load(name, grp):
            return op("sync", None, lambda t=t_in[name], s=inp[name]:
                      nc.sync.dma_start(out=t[:], in_=s[:]), grp)

        for name in G_A:
            load(name, "dina")
        dina_all = cnt["dina"]
        for name in G_L:
            load(name, "dinl")
        dinl_all = cnt["dinl"]
        load("pk16", "dinb")
        dinb_all = cnt["dinb"]
        load("pk32", "dinc")
        load("stind", "dinc")
        op("sync", None, lambda: nc.sync.dma_start(out=masks[0][:],
                                                   in_=inp["ttmask16"][0:128, :]), "dinc")
        op("sync", None, lambda: nc.sync.dma_start(out=masks[1][:],
                                                   in_=inp["ttmask16"][128:TS, :]), "dinc")
        dinc_all = cnt["dinc"]

        # ACT: load the tanh table right away; DVE: constant tiles
        act(None, lambda: act_i(scr[:], scr[:], AF.Tanh))
        d_ones = dve(None, lambda: V.memset(ones[:], 1.0))
        dve(None, lambda: V.memset(stL[0:1, :], 1.0))
        dve(None, lambda: V.memset(tsR[0:1, :], 1.0))

        # ================= PREP A: pu [64,2048] + sv [1,2048] ===============
        with nc.psum_tensor("pu", [2 * D, N], F32) as pu, \
             nc.psum_tensor("sv", [1, N], F32) as sv:
            for c in range(4):
                g_pu = pe([("dina", dina_all)] if c == 0 else None,
                          lambda c=c: mm(pu[:, c * 512:(c + 1) * 512], t_in["W96_R"][:],
                                         t_in["sp96"][:, c * 512:(c + 1) * 512],
                                         start=True, stop=True), "pe" if c == 3 else None)
            a_Rf = act([("pe", g_pu)], lambda: act_i(Rf16[:], pu[:], AF.Tanh, scale=3.0))

            # Lf right behind (only needs a_Rf + its own inputs)
            for c in range(2):
                g_pv = pe([("act", a_Rf), ("dinl", dinl_all)] if c == 0 else None,
                          lambda c=c: mm(pu[:, c * 512:(c + 1) * 512], t_in["W96_L"][:],
                                         t_in["sp96r"][:, c * 512:(c + 1) * 512],
                                         start=True, stop=True), "pe" if c == 1 else None)
            a_Lf = act([("pe", g_pv)], lambda: act_i(Lf16[:], pu[:, 0:NS], AF.Tanh,
                                                     scale=3.0))

            # svec round 1 (fp16): s1 | s2 | s1t packed into sv columns
            pe([("dinb", dinb_all)], lambda: mm(sv[0:1, 0:512], wst_a,
                                                spTr16[:, 0:512], start=True, stop=True))
            pe(None, lambda: mm(sv[0:1, 512:1024], wst_a,
                                spTr16[:, 512:1024], start=True, stop=True))
            pe(None, lambda: mm(sv[0:1, 1024:1024 + T], wst_b,
                                tmT16, start=True, stop=True))
            g_sv1 = pe(None, lambda: mm(sv[0:1, 1312:1312 + TS], wts_a,
                                        tmTr16, start=True, stop=True), "pe")

            d_s1 = dve([("pe", g_sv1), ("dinc", dinc_all)],
                       lambda: V.tensor_copy(s1row[:], sv[0:1, 0:NS]))
            d_s2 = dve(None, lambda: V.tensor_scalar_add(s2row[:], sv[0:1, 1024:1024 + T],
                                                         bst_ap))
            d_s1t = dve(None, lambda: V.tensor_copy(tsL[0:1, :], sv[0:1, 1312:1312 + TS]))

            # svec round 2: s2t over all N (overwrites sv)
            for c in range(4):
                g_sv2 = pe([("dve", d_s1t)] if c == 0 else None,
                           lambda c=c: mm(sv[0:1, c * 512:(c + 1) * 512], wts_b,
                                          spT16[:, c * 512:(c + 1) * 512],
                                          start=True, stop=True), "pe" if c == 3 else None)
            # split the psum->sbuf copy: bank 4 (cols 0:512) first, it gates stp
            d_s2t_a = dve([("pe", g_sv2)], lambda: V.tensor_scalar_add(
                s2tb[0:1, 0:512], sv[0:1, 0:512], bts_ap))
            d_s2t = dve(None, lambda: V.tensor_scalar_add(
                s2tb[0:1, 512:N], sv[0:1, 512:N], bts_ap))

        # aux DMAs (all overlap later compute)
        x_s1 = op("sync", [("dve", d_s1)],
                  lambda: nc.sync.dma_start(out=stL[1:9, :], in_=s1row[:]), "dmx")
        op("sync", [("dve", d_ones)],
           lambda: nc.sync.dma_start(out=tsL[1:2, :], in_=ones[0:1, 0:TS]), "dmx")
        x_tsr = op("sync", [("dve", d_s2t)],
                   lambda: nc.sync.dma_start(out=tsR[1:2, :], in_=s2tb[:]), "dmx")

        # ====== PREP B: stp [128, 2304] + ttp [128, 288] (tt done early) ====
        with nc.psum_tensor("stp", [128, NCHUNK * T], F32) as stp, \
             nc.psum_tensor("ttp", [128, T], F32) as ttp:
            npc = NCHUNK * T
            stw = [("act", a_Lf), ("dve", d_s2t_a), ("dmx", x_s1), ("dinc", dinc_all)]
            for c in range(5):
                c0, c1 = c * 512, min((c + 1) * 512, npc)
                pe(stw if c == 0 else None,
                   lambda c0=c0, c1=c1: mm(stp[:, c0:c1], stL[:],
                                           t_in["stind"][:, c0:c1],
                                           start=True, stop=False))
            for k in range(NCHUNK):
                g_stp = pe(None,
                           lambda k=k: mm(stp[:, k * T:(k + 1) * T], ones[0:1, 0:128],
                                          s2row[:], start=False, stop=True),
                           "pe" if k == NCHUNK - 1 else None)
            a_yst = act([("pe", g_stp)], lambda: act_i(y_st[:], stp[:], AF.Tanh))

            # tt block, fully staged into ttres during prep
            g_tt0 = pe([("dve", d_s2t)], lambda: mm(ttp[0:128, :], tmTr32[:, 0:128],
                                                    tmT32, start=True, stop=True), "pe")
            a_tt0 = act([("pe", g_tt0)], lambda: act_i(tttb[0:128, :], ttp[0:128, :],
                                                       AF.Tanh))
            tn = TS - 128
            g_tt1 = pe([("act", a_tt0)],
                       lambda tn=tn: mm(ttp[0:tn, :], tmTr32[:, 128:TS],
                                        tmT32, start=True, stop=True), "pe")
            # ttres = max(tanh,0) * triu-mask, one fused DVE op each
            d_tt0 = dve([("act", a_tt0), ("dinc", dinc_all)],
                        lambda: V.scalar_tensor_tensor(ttres[0][:], tttb[0:128, :], 0.0,
                                                       masks[0][:], ALU.max, ALU.mult))
            a_tt1 = act([("pe", g_tt1), ("dve", d_tt0)],
                        lambda tn=tn: act_i(tttb[0:tn, :], ttp[0:tn, :], AF.Tanh))
            dve([("act", a_tt1)],
                lambda tn=tn: V.scalar_tensor_tensor(ttres[1][:], tttb[0:tn, :], 0.0,
                                                     masks[1][:], ALU.max, ALU.mult))

        # ================= MAIN: zA + zB [128, 2048] ========================
        with nc.psum_tensor("zA", [128, N], F32) as zA, \
             nc.psum_tensor("zB", [128, N], F32) as zB:
            zps = [zA, zB]
            zact, gyr, dmul, dout_i, relu_d = [], [], [], [], []

            for i in range(NCHUNK):
                rs = slice(i * 128, (i + 1) * 128)
                zw = [("act", a_yst)] if i < 2 else [("act", zact[i - 2])]
                for c in range(4):
                    g_z = pe(zw if c == 0 else None,
                             lambda i=i, c=c: mm(zps[i % 2][:, c * 512:(c + 1) * 512],
                                                 Lf16[:, i * 128:(i + 1) * 128],
                                                 Rf16[:, c * 512:(c + 1) * 512],
                                                 start=True, stop=True),
                             "pe" if c == 3 else None)

                yw = [("pe", g_z)] + ([("gps", gyr[i - 3])] if i >= 3 else [])
                zact.append(act(yw, lambda i=i: act_i(ybufs[i % 3][:], zps[i % 2][:],
                                                      AF.Tanh, scale=AA)))

                # yr on the idle GPSIMD engine; yrbs ping-pong (dve mul frees it)
                gw = [("act", zact[i])] + ([("dve", dmul[i - 2])] if i >= 2 else [])
                gyr.append(gps(gw, lambda i=i: G.tensor_scalar_max(
                    yrbs[i % 2][:], ybufs[i % 3][:], 0.0)))

                dve([("gps", gyr[i])], lambda i=i: V.tensor_scalar(
                    wb[:], yrbs[i % 2][:], DA, CA, ALU.mult, ALU.add))
                ow = [(f"dout{i % 3}", dout_i[i - 3])] if i >= 3 else [("act", a_yst)]
                dmul.append(dve(ow, lambda i=i: V.tensor_mul(
                    outbufs[i % 3][:, 0:N], wb[:], yrbs[i % 2][:])))
                relu_d.append(dve(None, lambda i=i: V.tensor_scalar_max(
                    outbufs[i % 3][:, N:NT], y_st[:, i * T:(i + 1) * T], 0.0)))
                dout_i.append(op("sync", [("dve", relu_d[i])],
                                 lambda i=i, rs=rs: nc.sync.dma_start(
                                     out=out[rs, :], in_=outbufs[i % 3][:]),
                                 f"dout{i % 3}"))

            # ---- temporal ts rows; tt already staged in ttres ----
            # ts0 into zA (free after zact[6]), runs during zact[7]
            for c in range(4):
                g_ts0 = pe([("act", zact[6]), ("dmx", x_tsr)] if c == 0 else None,
                           lambda c=c: mm(zA[0:128, c * 512:(c + 1) * 512],
                                          tsL[:, 0:128], tsR[:, c * 512:(c + 1) * 512],
                                          start=True, stop=True),
                           "pe" if c == 3 else None)
            # ts1 into zB (free after zact[7])
            tn = TS - 128
            for c in range(4):
                g_ts1 = pe([("act", zact[7])] if c == 0 else None,
                           lambda c=c, tn=tn: mm(zB[0:tn, c * 512:(c + 1) * 512],
                                                 tsL[:, 128:TS],
                                                 tsR[:, c * 512:(c + 1) * 512],
                                                 start=True, stop=True),
                           "pe" if c == 3 else None)

            r = NCHUNK
            a_ts0 = act([("pe", g_ts0)],
                        lambda: act_i(ytb[0:128, :], zA[0:128, :], AF.Tanh))
            ow = [(f"dout{r % 3}", dout_i[r - 3]), ("act", a_ts0)]
            d_tsr0 = dve(ow, lambda r=r: V.tensor_scalar_max(
                outbufs[r % 3][0:128, 0:N], ytb[0:128, :], 0.0))
            relu_d.append(dve(None, lambda r=r: V.tensor_copy(
                outbufs[r % 3][0:128, N:NT], ttres[0][:])))
            dout_i.append(op("sync", [("dve", relu_d[r])],
                             lambda r=r: nc.sync.dma_start(
                                 out=out[NS:NS + 128, :], in_=outbufs[r % 3][0:128, :]),
                             f"dout{r % 3}"))

            r = NCHUNK + 1
            a_ts1 = act([("pe", g_ts1), ("dve", d_tsr0)],
                        lambda tn=tn: act_i(ytb[0:tn, :], zB[0:tn, :], AF.Tanh))
            ow = [(f"dout{r % 3}", dout_i[r - 3]), ("act", a_ts1)]
            dve(ow, lambda tn=tn, r=r: V.tensor_scalar_max(
                outbufs[r % 3][0:tn, 0:N], ytb[0:tn, :], 0.0))
            relu_d.append(dve(None, lambda tn=tn, r=r: V.tensor_copy(
                outbufs[r % 3][0:tn, N:NT], ttres[1][:])))
            dout_i.append(op("sync", [("dve", relu_d[r])],
                             lambda tn=tn, r=r: nc.sync.dma_start(
                                 out=out[NS + 128:ROWS, :], in_=outbufs[r % 3][0:tn, :]),
                             f"dout{r % 3}"))

        # ---------- emit ----------
        with nc.Block() as block:
            def make_body(engine_name):
                ops = plan[engine_name]

                def body(eng):
                    satisfied = {}
                    for waits, fn, inc in ops:
                        for sem_name, val in waits:
                            if val is not None and satisfied.get(sem_name, -1) < val:
                                eng.wait_ge(SEM[sem_name], val)
                                satisfied[sem_name] = val
                        ins = fn()
                        if inc is None:
                            continue
                        if inc in ("pe", "act", "dve", "gps"):
                            ins.then_inc(SEM[inc], 1)
                        else:
                            ins.then_inc(SEM[inc], 16)
                return body

            block.sync(make_body("sync"))
            block.tensor(make_body("tensor"))
            block.scalar(make_body("scalar"))
            block.vector(make_body("vector"))
            block.gpsimd(make_body("gpsimd"))

    return nc


def _bf16(x):
    u = x.astype(np.float32).view(np.uint32)
    r = ((u >> 16) + ((u >> 15) & 1)).astype(np.uint32) << 16
    return r.view(np.float32)


def build_in_maps(spatial_nodes, temporal_nodes, W_ss1, W_ss2, w_st, b_st, w_ts, b_ts):
    import ml_dtypes
    f, h = np.float32, np.float16
    bf = ml_dtypes.bfloat16

    def stack96(a32):
        hi = _bf16(a32)
        lo = _bf16(a32 - hi)
        return np.ascontiguousarray(np.concatenate([hi, lo, hi], axis=0)).astype(bf)

    def stackW(w32):
        hi = _bf16(w32)
        lo = _bf16(w32 - hi)
        return np.ascontiguousarray(np.concatenate([hi, hi, lo], axis=0)).astype(bf)

    W_R = np.concatenate([W_ss2.T, W_ss1.T], axis=1).astype(f)
    W_L = np.concatenate([W_ss1.T, -W_ss2.T], axis=1).astype(f)
    W96_R = stackW(W_R)
    W96_L = stackW(W_L)
    stind = np.zeros((9, NCHUNK * T), dtype=h)
    for k in range(NCHUNK):
        stind[k + 1, k * T:(k + 1) * T] = 1.0
    in_maps = []
    for c in range(N_CORES):
        b, hh = divmod(c, 2)
        tmask = (np.arange(T)[None, :] >= (hh * TS + np.arange(TS))[:, None]).astype(h)
        spT = np.ascontiguousarray(spatial_nodes[b].T, dtype=f)
        tmT = np.ascontiguousarray(temporal_nodes[b].T, dtype=f)
        spTr = np.ascontiguousarray(spT[:, hh * NS:(hh + 1) * NS])
        tmTr = np.ascontiguousarray(tmT[:, hh * TS:(hh + 1) * TS])
        sp96 = stack96(spT)
        pk16 = np.zeros((D, PK_W16), dtype=h)
        pk16[:, PK_SPT:PK_SPT + N] = spT
        pk16[:, PK_SPTR:PK_SPTR + NS] = spTr
        pk16[:, PK_TMT:PK_TMT + T] = tmT
        pk16[:, PK_TMTR:PK_TMTR + TS] = tmTr
        pk16[:, PK_W] = w_st[:D]
        pk16[:, PK_W + 1] = w_st[D:]
        pk16[:, PK_W + 2] = w_ts[:D]
        pk16[:, PK_W + 3] = w_ts[D:]
        pk32 = np.zeros((D, T + TS + 2), dtype=f)
        pk32[:, 0:T] = tmT
        pk32[:, T:T + TS] = tmTr
        pk32[0, T + TS] = b_st
        pk32[0, T + TS + 1] = b_ts
        in_maps.append({
            "sp96": sp96, "W96_R": W96_R,
            "sp96r": np.ascontiguousarray(sp96[:, hh * NS:(hh + 1) * NS]),
            "W96_L": W96_L,
            "pk16": pk16, "pk32": np.ascontiguousarray(pk32),
            "ttmask16": tmask,
            "stind": stind,
        })
    return in_maps


def assemble(results):
    out = np.empty((B, NT, NT), np.float32)
    for c in range(N_CORES):
        b, h = divmod(c, 2)
        r = results[c]["out"].astype(np.float32)
        out[b, h * NS:(h + 1) * NS, :] = r[0:NS]
        out[b, N + h * TS: N + (h + 1) * TS, :] = r[NS:ROWS]
    return out


_NC = None


def kernel(**inputs):
    global _NC
    if _NC is None:
        _NC = build_program()
    in_maps = build_in_maps(**inputs)
    res = run_bass_kernel_spmd(_NC, in_maps, list(range(N_CORES)))
    return assemble(res.results)


# revision 36
# speedup vs baseline: 4.6299x; 4.6299x over previous
"""Trainium2 Bass kernel for the MLPSim adjacency-constructor problem.

Full shapes: spatial [4, 2048, 32], temporal [4, 288, 32], output
adj [4, 2336, 2336] f32 where adj = tanh(relu(blocks)):
  ss = tanh(m - m^T), m = nv1 @ nv2^T, nv_i = tanh(3*x@W_i^T)
  st = s1[n] + s2[t] + b_st ;  ts = s1t[t] + s2t[n] + b_ts
  tt = triu(temporal @ temporal^T)

Sharding: 8 cores = (batch b = c//2) x (row-half h = c%2); each core emits
1024 spatial + 144 temporal rows ([1168, 2336]) of one batch, stored fp16
on device and upcast to f32 on the host during unshard.

Device algebra (ACT is the wall: 1 elem/lane/cycle, dtype-independent):
  ss out = tanh(tanh(relu(z))), z = m - m^T. Minimax fit
    tanh(tanh(relu(z))) ~= max((C + D*y)*y, 0),  y = tanh(A*z)
  with A=1.15118303 C=0.90136458 D=-0.141975 (fit err 2.2e-3; (C+D*y)*y is
  negative whenever y<0, so the final relu also zeroes the z<0 half) folds
  the two full-size ACT tanh passes into ONE; the quadratic runs on DVE in
  3 fused 16-bit ops. z is a plain fp16 matmul (nv exact, one fp16
  rounding -> ~7e-3 total err vs the 2e-2 gate). nv pre-acts u = x@W are
  an exact bf16 hi/lo K-stacked matmul ([Wh;Wh;Wl].T @ [xh;xl;xh], err
  ~1e-5), avoiding slow f32 matmuls. st is a K=9 fp16 matmul (chunk-
  indicator rows) plus rank-1 s2 accumulate-matmuls + ONE ACT pass for
  all 8 chunks; ts is a K=2 rank-1 fp16 matmul; tt stays f32 (288 cols).
  Temporal passes interleave into the tail of the spatial ACT stream.
"""

import numpy as np
from contextlib import ExitStack

import concourse.bass as bass
from concourse import mybir
from concourse.bass_utils import run_bass_kernel_spmd

AF = mybir.ActivationFunctionType
ALU = mybir.AluOpType
F32 = mybir.dt.float32
F16 = mybir.dt.float16
BF16 = mybir.dt.bfloat16

B, N, T, D = 4, 2048, 288, 32
NS = N // 2
TS = T // 2
NT = N + T
ROWS = NS + TS
N_CORES = 8
NCHUNK = NS // 128

AA = 1.15118303
CA = 0.90136458
DA = -0.141975

G_A = ("sp96", "W96_R")
G_L = ("sp96r", "W96_L")

# packed fp16 blob layout (columns)
PK_SPT = 0            # spT16 full [32, 2048]
PK_SPTR = 2048        # spTr16 rows-half [32, 1024]
PK_TMT = 3072         # tmT16 [32, 288]
PK_TMTR = 3360        # tmTr16 [32, 144]
PK_W = 3504           # wst_a | wst_b | wts_a | wts_b, one col each
PK_B = 3508           # bst at [0, 3508], bts at [0, 3509]
PK_W16 = 3510


def build_program():
    nc = bass.Bass()
    inp = {}

    for name, shape, dt in (
        ("sp96", (3 * D, N), BF16), ("W96_R", (3 * D, 2 * D), BF16),
        ("sp96r", (3 * D, NS), BF16), ("W96_L", (3 * D, 2 * D), BF16),
        ("pk16", (D, PK_W16), F16), ("pk32", (D, T + TS + 2), F32),
        ("ttmask16", (TS, T), F16), ("stind", (9, NCHUNK * T), F16),
    ):
        inp[name] = nc.declare_dram_parameter(name, list(shape), dt, isOutput=False)
    out = nc.declare_dram_parameter("out", [ROWS, NT], F16, isOutput=True)

    ctx = ExitStack()
    _uid = [0]

    def sbuf(shape, dt=F16):
        _uid[0] += 1
        return ctx.enter_context(nc.sbuf_tensor(f"sb{_uid[0]}", list(shape), dt))

    with ctx:
        t_in = {k: sbuf(v.shape, v.dtype) for k, v in inp.items() if k != "ttmask16"}
        masks = [sbuf([128, T]), sbuf([TS - 128, T])]
        Rf16 = sbuf([2 * D, N])
        Lf16 = sbuf([2 * D, NS])
        s2row = sbuf([1, T])
        s2tb = sbuf([1, N])
        ones = sbuf([1, N])
        stL = sbuf([9, 128])
        s1row = sbuf([1, NS])
        tsL = sbuf([2, TS])           # [s1t ; ones]
        tsR = sbuf([2, N])            # [ones ; s2t+b]
        y_st = sbuf([128, NCHUNK * T])
        ybufs = [sbuf([128, N]) for _ in range(3)]
        yrbs = [sbuf([128, N]) for _ in range(2)]
        wb = sbuf([128, N])
        ytb = sbuf([128, N])
        tttb = sbuf([128, T])
        ttres = [sbuf([128, T]), sbuf([TS - 128, T])]
        scr = sbuf([1, 8], F32)
        outbufs = [sbuf([128, NT]) for _ in range(3)]

        pk = t_in["pk16"]
        spT16 = pk[:, PK_SPT:PK_SPT + N]
        spTr16 = pk[:, PK_SPTR:PK_SPTR + NS]
        tmT16 = pk[:, PK_TMT:PK_TMT + T]
        tmTr16 = pk[:, PK_TMTR:PK_TMTR + TS]
        wst_a = pk[:, PK_W:PK_W + 1]
        wst_b = pk[:, PK_W + 1:PK_W + 2]
        wts_a = pk[:, PK_W + 2:PK_W + 3]
        wts_b = pk[:, PK_W + 3:PK_W + 4]
        tmT32 = t_in["pk32"][:, 0:T]
        tmTr32 = t_in["pk32"][:, T:T + TS]
        bst_ap = t_in["pk32"][0:1, T + TS:T + TS + 1]
        bts_ap = t_in["pk32"][0:1, T + TS + 1:T + TS + 2]

        sems = {}
        for sname in ("dina", "dinl", "dinb", "dinc", "dmx", "pe_s", "act_s",
                      "dve_s", "gps_s", "dout0", "dout1", "dout2"):
            sems[sname] = ctx.enter_context(nc.semaphore(sname))
        SEM = {"pe": sems["pe_s"], "act": sems["act_s"], "dve": sems["dve_s"],
               "gps": sems["gps_s"],
               "dina": sems["dina"], "dinl": sems["dinl"], "dinb": sems["dinb"],
               "dinc": sems["dinc"], "dmx": sems["dmx"],
               "dout0": sems["dout0"], "dout1": sems["dout1"], "dout2": sems["dout2"]}

        plan = {"sync": [], "tensor": [], "scalar": [], "vector": [], "gpsimd": []}
        cnt = {k: 0 for k in SEM}

        def op(engine, waits, fn, inc=None):
            plan[engine].append((waits or [], fn, inc))
            if inc:
                cnt[inc] += 1 if inc in ("pe", "act", "dve", "gps") else 16
                return cnt[inc]
            return None

        def pe(waits, fn, inc=None):
            return op("tensor", waits, fn, inc)

        def act(waits, fn):
            return op("scalar", waits, fn, "act")

        def dve(waits, fn):
            return op("vector", waits, fn, "dve")

        def gps(waits, fn):
            return op("gpsimd", waits, fn, "gps")

        mm = nc.tensor.matmul
        act_i = nc.scalar.activation
        V = nc.vector
        G = nc.gpsimd

        # ---------- input loads, finest-grained gating first ----------
        def load(name, grp):
            return op("sync", None, lambda t=t_in[name], s=inp[name]:
                      nc.sync.dma_start(out=t[:], in_=s[:]), grp)

        for name in G_A:
            load(name, "dina")
        dina_all = cnt["dina"]
        for name in G_L:
            load(name, "dinl")
        dinl_all = cnt["dinl"]
        load("pk16", "dinb")
        dinb_all = cnt["dinb"]
        load("pk32", "dinc")
        load("stind", "dinc")
        op("sync", None, lambda: nc.sync.dma_start(out=masks[0][:],
                                                   in_=inp["ttmask16"][0:128, :]), "dinc")
        op("sync", None, lambda: nc.sync.dma_start(out=masks[1][:],
                                                   in_=inp["ttmask16"][128:TS, :]), "dinc")
        dinc_all = cnt["dinc"]

        # ACT: load the tanh table right away; DVE: constant tiles
        act(None, lambda: act_i(scr[:], scr[:], AF.Tanh))
        d_ones = dve(None, lambda: V.memset(ones[:], 1.0))
        dve(None, lambda: V.memset(stL[0:1, :], 1.0))
        dve(None, lambda: V.memset(tsR[0:1, :], 1.0))

        # ================= PREP A: pu [64,2048] + sv [1,2048] ===============
        with nc.psum_tensor("pu", [2 * D, N], F32) as pu, \
             nc.psum_tensor("sv", [1, N], F32) as sv:
            for c in range(4):
                g_pu = pe([("dina", dina_all)] if c == 0 else None,
                          lambda c=c: mm(pu[:, c * 512:(c + 1) * 512], t_in["W96_R"][:],
                                         t_in["sp96"][:, c * 512:(c + 1) * 512],
                                         start=True, stop=True), "pe" if c == 3 else None)
            a_Rf = act([("pe", g_pu)], lambda: act_i(Rf16[:], pu[:], AF.Tanh, scale=3.0))

            # Lf right behind (only needs a_Rf + its own inputs)
            for c in range(2):
                g_pv = pe([("act", a_Rf), ("dinl", dinl_all)] if c == 0 else None,
                          lambda c=c: mm(pu[:, c * 512:(c + 1) * 512], t_in["W96_L"][:],
                                         t_in["sp96r"][:, c * 512:(c + 1) * 512],
                                         start=True, stop=True), "pe" if c == 1 else None)
            a_Lf = act([("pe", g_pv)], lambda: act_i(Lf16[:], pu[:, 0:NS], AF.Tanh,
                                                     scale=3.0))

            # svec round 1 (fp16): s1 | s2 | s1t packed into sv columns
            pe([("dinb", dinb_all)], lambda: mm(sv[0:1, 0:512], wst_a,
                                                spTr16[:, 0:512], start=True, stop=True))
            pe(None, lambda: mm(sv[0:1, 512:1024], wst_a,
                                spTr16[:, 512:1024], start=True, stop=True))
            pe(None, lambda: mm(sv[0:1, 1024:1024 + T], wst_b,
                                tmT16, start=True, stop=True))
            g_sv1 = pe(None, lambda: mm(sv[0:1, 1312:1312 + TS], wts_a,
                                        tmTr16, start=True, stop=True), "pe")

            d_s1 = dve([("pe", g_sv1), ("dinc", dinc_all)],
                       lambda: V.tensor_copy(s1row[:], sv[0:1, 0:NS]))
            d_s2 = dve(None, lambda: V.tensor_scalar_add(s2row[:], sv[0:1, 1024:1024 + T],
                                                         bst_ap))
            d_s1t = dve(None, lambda: V.tensor_copy(tsL[0:1, :], sv[0:1, 1312:1312 + TS]))

            # svec round 2: s2t over all N (overwrites sv)
            for c in range(4):
                g_sv2 = pe([("dve", d_s1t)] if c == 0 else None,
                           lambda c=c: mm(sv[0:1, c * 512:(c + 1) * 512], wts_b,
                                          spT16[:, c * 512:(c + 1) * 512],
                                          start=True, stop=True), "pe" if c == 3 else None)
            # split the psum->sbuf copy: bank 4 (cols 0:512) first, it gates stp
            d_s2t_a = dve([("pe", g_sv2)], lambda: V.tensor_scalar_add(
                s2tb[0:1, 0:512], sv[0:1, 0:512], bts_ap))
            d_s2t = dve(None, lambda: V.tensor_scalar_add(
                s2tb[0:1, 512:N], sv[0:1, 512:N], bts_ap))

        # aux DMAs (all overlap later compute)
        x_s1 = op("sync", [("dve", d_s1)],
                  lambda: nc.sync.dma_start(out=stL[1:9, :], in_=s1row[:]), "dmx")
        op("sync", [("dve", d_ones)],
           lambda: nc.sync.dma_start(out=tsL[1:2, :], in_=ones[0:1, 0:TS]), "dmx")
        x_tsr = op("sync", [("dve", d_s2t)],
                   lambda: nc.sync.dma_start(out=tsR[1:2, :], in_=s2tb[:]), "dmx")

        # ====== PREP B: stp [128, 2304] + ttp [128, 288] (tt done early) ====
        with nc.psum_tensor("stp", [128, NCHUNK * T], F32) as stp, \
             nc.psum_tensor("ttp", [128, T], F32) as ttp:
            npc = NCHUNK * T
            stw = [("act", a_Lf), ("dve", d_s2t_a), ("dmx", x_s1), ("dinc", dinc_all)]
            for c in range(5):
                c0, c1 = c * 512, min((c + 1) * 512, npc)
                pe(stw if c == 0 else None,
                   lambda c0=c0, c1=c1: mm(stp[:, c0:c1], stL[:],
                                           t_in["stind"][:, c0:c1],
                                           start=True, stop=False))
            for k in range(NCHUNK):
                g_stp = pe(None,
                           lambda k=k: mm(stp[:, k * T:(k + 1) * T], ones[0:1, 0:128],
                                          s2row[:], start=False, stop=True),
                           "pe" if k == NCHUNK - 1 else None)
            a_yst = act([("pe", g_stp)], lambda: act_i(y_st[:], stp[:], AF.Tanh))

            # tt block, fully staged into ttres during prep
            g_tt0 = pe([("dve", d_s2t)], lambda: mm(ttp[0:128, :], tmTr32[:, 0:128],
                                                    tmT32, start=True, stop=True), "pe")
            a_tt0 = act([("pe", g_tt0)], lambda: act_i(tttb[0:128, :], ttp[0:128, :],
                                                       AF.Tanh))
            tn = TS - 128
            g_tt1 = pe([("act", a_tt0)],
                       lambda tn=tn: mm(ttp[0:tn, :], tmTr32[:, 128:TS],
                                        tmT32, start=True, stop=True), "pe")
            # ttres = max(tanh,0) * triu-mask, one fused DVE op each
            d_tt0 = dve([("act", a_tt0), ("dinc", dinc_all)],
                        lambda: V.scalar_tensor_tensor(ttres[0][:], tttb[0:128, :], 0.0,
                                                       masks[0][:], ALU.max, ALU.mult))
            a_tt1 = act([("pe", g_tt1), ("dve", d_tt0)],
                        lambda tn=tn: act_i(tttb[0:tn, :], ttp[0:tn, :], AF.Tanh))
            dve([("act", a_tt1)],
                lambda tn=tn: V.scalar_tensor_tensor(ttres[1][:], tttb[0:tn, :], 0.0,
                                                     masks[1][:], ALU.max, ALU.mult))

        # ================= MAIN: zA + zB [128, 2048] ========================
        with nc.psum_tensor("zA", [128, N], F32) as zA, \
             nc.psum_tensor("zB", [128, N], F32) as zB:
            zps = [zA, zB]
            zact, gyr, dmul, dout_i, relu_d = [], [], [], [], []

            for i in range(NCHUNK):
                rs = slice(i * 128, (i + 1) * 128)
                zw = [("act", a_yst)] if i < 2 else [("act", zact[i - 2])]
                for c in range(4):
                    g_z = pe(zw if c == 0 else None,
                             lambda i=i, c=c: mm(zps[i % 2][:, c * 512:(c + 1) * 512],
                                                 Lf16[:, i * 128:(i + 1) * 128],
                                                 Rf16[:, c * 512:(c + 1) * 512],
                                                 start=True, stop=True),
                             "pe" if c == 3 else None)

                yw = [("pe", g_z)] + ([("dve", gyr[i - 3])] if i >= 3 else [])
                zact.append(act(yw, lambda i=i: act_i(ybufs[i % 3][:], zps[i % 2][:],
                                                      AF.Tanh, scale=AA)))

                gyr.append(dve([("act", zact[i])], lambda i=i: V.tensor_scalar_max(
                    yrbs[i % 2][:], ybufs[i % 3][:], 0.0)))
                dve(None, lambda i=i: V.tensor_scalar(
                    wb[:], yrbs[i % 2][:], DA, CA, ALU.mult, ALU.add))
                ow = [(f"dout{i % 3}", dout_i[i - 3])] if i >= 3 else [("act", a_yst)]
                dmul.append(dve(ow, lambda i=i: V.tensor_mul(
                    outbufs[i % 3][:, 0:N], wb[:], yrbs[i % 2][:])))
                relu_d.append(dve(None, lambda i=i: V.tensor_scalar_max(
                    outbufs[i % 3][:, N:NT], y_st[:, i * T:(i + 1) * T], 0.0)))
                dout_i.append(op("sync", [("dve", relu_d[i])],
                                 lambda i=i, rs=rs: nc.sync.dma_start(
                                     out=out[rs, :], in_=outbufs[i % 3][:]),
                                 f"dout{i % 3}"))

            # ---- temporal ts rows; tt already staged in ttres ----
            # ts0 into zA (free after zact[6]), runs during zact[7]
            for c in range(4):
                g_ts0 = pe([("act", zact[6]), ("dmx", x_tsr)] if c == 0 else None,
                           lambda c=c: mm(zA[0:128, c * 512:(c + 1) * 512],
                                          tsL[:, 0:128], tsR[:, c * 512:(c + 1) * 512],
                                          start=True, stop=True),
                           "pe" if c == 3 else None)
            # ts1 into zB (free after zact[7])
            tn = TS - 128
            for c in range(4):
                g_ts1 = pe([("act", zact[7])] if c == 0 else None,
                           lambda c=c, tn=tn: mm(zB[0:tn, c * 512:(c + 1) * 512],
                                                 tsL[:, 128:TS],
                                                 tsR[:, c * 512:(c + 1) * 512],
                                                 start=True, stop=True),
                           "pe" if c == 3 else None)

            r = NCHUNK
            a_ts0 = act([("pe", g_ts0)],
                        lambda: act_i(ytb[0:128, :], zA[0:128, :], AF.Tanh))
            ow = [(f"dout{r % 3}", dout_i[r - 3]), ("act", a_ts0)]
            d_tsr0 = dve(ow, lambda r=r: V.tensor_scalar_max(
                outbufs[r % 3][0:128, 0:N], ytb[0:128, :], 0.0))
            relu_d.append(dve(None, lambda r=r: V.tensor_copy(
                outbufs[r % 3][0:128, N:NT], ttres[0][:])))
            dout_i.append(op("sync", [("dve", relu_d[r])],
                             lambda r=r: nc.sync.dma_start(
                                 out=out[NS:NS + 128, :], in_=outbufs[r % 3][0:128, :]),
                             f"dout{r % 3}"))

            r = NCHUNK + 1
            a_ts1 = act([("pe", g_ts1), ("dve", d_tsr0)],
                        lambda tn=tn: act_i(ytb[0:tn, :], zB[0:tn, :], AF.Tanh))
            ow = [(f"dout{r % 3}", dout_i[r - 3]), ("act", a_ts1)]
            dve(ow, lambda tn=tn, r=r: V.tensor_scalar_max(
                outbufs[r % 3][0:tn, 0:N], ytb[0:tn, :], 0.0))
            relu_d.append(dve(None, lambda tn=tn, r=r: V.tensor_copy(
                outbufs[r % 3][0:tn, N:NT], ttres[1][:])))
            dout_i.append(op("sync", [("dve", relu_d[r])],
                             lambda tn=tn, r=r: nc.sync.dma_start(
                                 out=out[NS + 128:ROWS, :], in_=outbufs[r % 3][0:tn, :]),
                             f"dout{r % 3}"))

        # ---------- emit ----------
        with nc.Block() as block:
            def make_body(engine_name):
                ops = plan[engine_name]

                def body(eng):
                    satisfied = {}
                    for waits, fn, inc in ops:
                        for sem_name, val in waits:
                            if val is not None and satisfied.get(sem_name, -1) < val:
                                eng.wait_ge(SEM[sem_name], val)
                                satisfied[sem_name] = val
                        ins = fn()
                        if inc is None:
                            continue
                        if inc in ("pe", "act", "dve", "gps"):
                            ins.then_inc(SEM[inc], 1)
                        else:
                            ins.then_inc(SEM[inc], 16)
                return body

            block.sync(make_body("sync"))
            block.tensor(make_body("tensor"))
            block.scalar(make_body("scalar"))
            block.vector(make_body("vector"))
            block.gpsimd(make_body("gpsimd"))

    return nc


def _bf16(x):
    u = x.astype(np.float32).view(np.uint32)
    r = ((u >> 16) + ((u >> 15) & 1)).astype(np.uint32) << 16
    return r.view(np.float32)


def build_in_maps(spatial_nodes, temporal_nodes, W_ss1, W_ss2, w_st, b_st, w_ts, b_ts):
    import ml_dtypes
    f, h = np.float32, np.float16
    bf = ml_dtypes.bfloat16

    def stack96(a32):
        hi = _bf16(a32)
        lo = _bf16(a32 - hi)
        return np.ascontiguousarray(np.concatenate([hi, lo, hi], axis=0)).astype(bf)

    def stackW(w32):
        hi = _bf16(w32)
        lo = _bf16(w32 - hi)
        return np.ascontiguousarray(np.concatenate([hi, hi, lo], axis=0)).astype(bf)

    W_R = np.concatenate([W_ss2.T, W_ss1.T], axis=1).astype(f)
    W_L = np.concatenate([W_ss1.T, -W_ss2.T], axis=1).astype(f)
    W96_R = stackW(W_R)
    W96_L = stackW(W_L)
    stind = np.zeros((9, NCHUNK * T), dtype=h)
    for k in range(NCHUNK):
        stind[k + 1, k * T:(k + 1) * T] = 1.0
    in_maps = []
    for c in range(N_CORES):
        b, hh = divmod(c, 2)
        tmask = (np.arange(T)[None, :] >= (hh * TS + np.arange(TS))[:, None]).astype(h)
        spT = np.ascontiguousarray(spatial_nodes[b].T, dtype=f)
        tmT = np.ascontiguousarray(temporal_nodes[b].T, dtype=f)
        spTr = np.ascontiguousarray(spT[:, hh * NS:(hh + 1) * NS])
        tmTr = np.ascontiguousarray(tmT[:, hh * TS:(hh + 1) * TS])
        sp96 = stack96(spT)
        pk16 = np.zeros((D, PK_W16), dtype=h)
        pk16[:, PK_SPT:PK_SPT + N] = spT
        pk16[:, PK_SPTR:PK_SPTR + NS] = spTr
        pk16[:, PK_TMT:PK_TMT + T] = tmT
        pk16[:, PK_TMTR:PK_TMTR + TS] = tmTr
        pk16[:, PK_W] = w_st[:D]
        pk16[:, PK_W + 1] = w_st[D:]
        pk16[:, PK_W + 2] = w_ts[:D]
        pk16[:, PK_W + 3] = w_ts[D:]
        pk32 = np.zeros((D, T + TS + 2), dtype=f)
        pk32[:, 0:T] = tmT
        pk32[:, T:T + TS] = tmTr
        pk32[0, T + TS] = b_st
        pk32[0, T + TS + 1] = b_ts
        in_maps.append({
            "sp96": sp96, "W96_R": W96_R,
            "sp96r": np.ascontiguousarray(sp96[:, hh * NS:(hh + 1) * NS]),
            "W96_L": W96_L,
            "pk16": pk16, "pk32": np.ascontiguousarray(pk32),
            "ttmask16": tmask,
            "stind": stind,
        })
    return in_maps


def assemble(results):
    out = np.empty((B, NT, NT), np.float32)
    for c in range(N_CORES):
        b, h = divmod(c, 2)
        r = results[c]["out"].astype(np.float32)
        out[b, h * NS:(h + 1) * NS, :] = r[0:NS]
        out[b, N + h * TS: N + (h + 1) * TS, :] = r[NS:ROWS]
    return out


_NC = None


def kernel(**inputs):
    global _NC
    if _NC is None:
        _NC = build_program()
    in_maps = build_in_maps(**inputs)
    res = run_bass_kernel_spmd(_NC, in_maps, list(range(N_CORES)))
    return assemble(res.results)
